# revision 16
# baseline (speedup 1.0000x reference)
"""Trainium2 Bass kernel for nn_CcLoss (gnn_message_passing).

Full inputs: features [64, 1024, 128] f32, tau scalar f32.
Data-parallel over batch B across 8 NeuronCores (8 samples per core).

Per sample b (on device):
  fn    = f / ||f||_rows                   (bf16)
  sim   = fn @ fn.T                        (PE bf16 -> fp32 PSUM)
  mask  : ScalarE rows use S=sign(sim-tau) in {-1,0,1}; VectorE rows use
          (sim>tau)-0.5 in {-0.5,+0.5}. Row sums (deg) fused via accum_out.
          No ACT table swaps: sign/sqrt/square/copy share one table.
  proto = (mask @ f)/deg via the identity M = (S+1)/2:
          proto_raw = S@(f/2) + colsum(f)/2   (K=1 rank-1 correction matmul)
  stats : Sum f^2 (per row-tile), Sum proto*f, Sum proto^2, gtsum[d]
Host combines stats into MSE + Pearson loss (exact algebra of the reference).
"""

import numpy as np

B, P, D = 64, 1024, 128
NCORES = 8
BLOC = B // NCORES          # samples per core
NT = P // 128               # 128-row tiles per sample
ROW = 160                   # per-sample stats row stride in the output
N_ACT = 6                   # compare row-tiles on ScalarE (rest on VectorE)

_PROG = None


def _build_program():
    import concourse.tile as tile
    from concourse import bacc, mybir, masks

    f32 = mybir.dt.float32
    bf16 = mybir.dt.bfloat16
    AF = mybir.ActivationFunctionType
    OP = mybir.AluOpType

    nc = bacc.Bacc(
        "TRN2",
        target_bir_lowering=False,
        debug=False,
        enable_asserts=False,
        num_devices=NCORES,
    )
    feats = nc.dram_tensor("features", [BLOC, P, D], f32, kind="ExternalInput").ap()
    tau_d = nc.dram_tensor("tau", [1, 1], f32, kind="ExternalInput").ap()
    out_d = nc.dram_tensor("out", [1, BLOC * ROW], f32, kind="ExternalOutput").ap()

    with tile.TileContext(nc) as tc:
        from contextlib import ExitStack

        with ExitStack() as ctx:
            const = ctx.enter_context(tc.tile_pool(name="const", bufs=1))
            fpool = ctx.enter_context(tc.tile_pool(name="f", bufs=2))
            fnpool = ctx.enter_context(tc.tile_pool(name="fn", bufs=2))
            fhpool = ctx.enter_context(tc.tile_pool(name="fh", bufs=2))
            fbpool = ctx.enter_context(tc.tile_pool(name="fb16", bufs=2))
            ftpool = ctx.enter_context(tc.tile_pool(name="fnT", bufs=2))
            mpool = ctx.enter_context(tc.tile_pool(name="mask", bufs=2))
            ppool = ctx.enter_context(tc.tile_pool(name="proto", bufs=2))
            stpool = ctx.enter_context(tc.tile_pool(name="stat", bufs=3))
            smpool = ctx.enter_context(tc.tile_pool(name="small", bufs=4))
            dscr = ctx.enter_context(tc.tile_pool(name="dscr", bufs=2))
            gscr = ctx.enter_context(tc.tile_pool(name="gscr", bufs=2))
            pss_pool = ctx.enter_context(tc.tile_pool(name="pss", bufs=2, space="PSUM"))
            pmm_pool = ctx.enter_context(tc.tile_pool(name="pmm", bufs=2, space="PSUM"))
            pg_pool = ctx.enter_context(tc.tile_pool(name="pg", bufs=1, space="PSUM"))
            pst_pool = ctx.enter_context(tc.tile_pool(name="pstat", bufs=1, space="PSUM"))

            ident16 = const.tile([128, 128], bf16)
            masks.make_identity(nc, ident16[:])
            ones = const.tile([128, 1], f32)
            nc.gpsimd.memset(ones[:], 1.0)
            ones16 = const.tile([128, 1], bf16)
            nc.gpsimd.memset(ones16[:], 1.0)
            onesrow16 = const.tile([1, 128], bf16)
            nc.gpsimd.memset(onesrow16[:], 1.0)
            tau_bc = const.tile([128, 1], f32)
            nc.sync.dma_start(tau_bc[:], tau_d[0, :].partition_broadcast(128))
            ntau = const.tile([128, 1], f32)
            nc.gpsimd.tensor_scalar_mul(ntau[:], tau_bc[:], -1.0)
            halves = const.tile([128, 1], f32)
            nc.gpsimd.memset(halves[:], 0.5)
            srow = const.tile([1, BLOC * ROW], f32)
            nc.gpsimd.memset(srow[:], 0.0)

            for s in range(BLOC):
                # ---- load sample as 8 [128,128] tiles packed in [128, 1024] ----
                fb = fpool.tile([128, NT * 128], f32, tag="fb")
                nc.sync.dma_start(
                    fb[:].rearrange("p (t d) -> p t d", t=NT),
                    feats[s].rearrange("(t p) d -> p t d", p=128),
                )

                statv = stpool.tile([128, 10], f32, tag="statv")

                # ---- row norms^2 -> statv[:, t] : ACT square + DVE reduce ----
                sq = dscr.tile([128, NT * 128], f32, tag="dscr")
                nc.scalar.activation(sq[:], fb[:], AF.Square)
                nc.vector.tensor_reduce(
                    statv[:, 0:8],
                    sq[:].rearrange("p (t d) -> p t d", t=NT),
                    axis=mybir.AxisListType.X,
                    op=OP.add,
                )
                sroot = smpool.tile([128, 8], f32, tag="sroot")
                nc.scalar.activation(sroot[:], statv[:, 0:8], AF.Sqrt)
                rinv = smpool.tile([128, 8], f32, tag="rinv")
                nc.vector.reciprocal(rinv[:], sroot[:])

                # ---- fn = f/||f|| (bf16, one broadcast op);
                #      fh = bf16(f/2); fv = bf16(f) ----
                fn = fnpool.tile([128, NT * 128], bf16, tag="fn")
                nc.vector.tensor_tensor(
                    fn[:].rearrange("p (t d) -> p t d", t=NT),
                    fb[:].rearrange("p (t d) -> p t d", t=NT),
                    rinv[:].unsqueeze(2).broadcast_to([128, NT, 128]),
                    op=OP.mult,
                )
                fh = fhpool.tile([128, NT * 128], bf16, tag="fh")
                nc.vector.tensor_scalar_mul(fh[:], fb[:], 0.5)
                fv = fbpool.tile([128, NT * 128], bf16, tag="fv")
                nc.vector.tensor_copy(fv[:], fb[:])

                # ---- colsum(f)/2 for the rank-1 proto correction ----
                psc = pst_pool.tile([128, 128], f32, tag="pstat")
                for kc in range(NT):
                    nc.tensor.matmul(
                        psc[0:1, :],
                        ones16[:, 0:1],
                        fh[:, kc * 128:(kc + 1) * 128],
                        start=(kc == 0),
                        stop=(kc == NT - 1),
                    )
                ch = smpool.tile([1, 128], bf16, tag="ch")
                nc.scalar.copy(ch[:], psc[0:1, :])
                chlo = smpool.tile([1, 128], bf16, tag="chlo")
                nc.vector.tensor_tensor(
                    chlo[:], psc[0:1, :], ch[:], op=OP.subtract
                )

                # ---- transpose fn -> fnT [D, P]; 4 transposes per PSUM bank ----
                fnT = ftpool.tile([128, P], bf16, tag="fnT")
                for h in range(2):
                    pst = pmm_pool.tile([128, 512], bf16, tag="mm512")
                    for q in range(4):
                        t = h * 4 + q
                        nc.tensor.matmul(
                            pst[:, q * 128:(q + 1) * 128],
                            fn[:, t * 128:(t + 1) * 128],
                            ident16[:],
                            is_transpose=True,
                        )
                    nc.scalar.copy(fnT[:, h * 512:(h + 1) * 512], pst[:])

                # ---- similarity + mask (+ fused deg accum), one op per mt ----
                mask_t = mpool.tile([128, NT * P], bf16, tag="mask")
                dacc = smpool.tile([128, 8], f32, tag="dacc")
                for mt in range(NT):
                    pss = pss_pool.tile([128, 1024], f32, tag="pss")
                    for nb in range(2):
                        nc.tensor.matmul(
                            pss[:, nb * 512:(nb + 1) * 512],
                            fnT[:, mt * 128:(mt + 1) * 128],
                            fnT[:, nb * 512:(nb + 1) * 512],
                            start=True,
                            stop=True,
                        )
                    blk = mask_t[:, mt * P:(mt + 1) * P]
                    acc = dacc[:, mt:mt + 1]
                    if mt < N_ACT:
                        # S = sign(sim - tau); accum = 2*deg - 1024
                        nc.scalar.activation(
                            blk, pss[:], AF.Sign,
                            bias=ntau[:], scale=1.0, accum_out=acc,
                        )
                    else:
                        # (sim>tau) - 0.5 = S/2; accum = deg - 512
                        nc.vector.scalar_tensor_tensor(
                            blk, pss[:], tau_bc[:],
                            halves[:, 0:1].broadcast_to([128, 1024]),
                            op0=OP.is_gt, op1=OP.subtract, accum_out=acc,
                        )

                # deg: ACT rows = 0.5*acc + 512 ; DVE rows = acc + 512
                degc = smpool.tile([128, 8], f32, tag="degc")
                nc.vector.tensor_scalar(
                    degc[:, 0:N_ACT], dacc[:, 0:N_ACT], 0.5, 512.0,
                    op0=OP.mult, op1=OP.add,
                )
                nc.vector.tensor_scalar_add(
                    degc[:, N_ACT:8], dacc[:, N_ACT:8], 512.0
                )
                rdeg = smpool.tile([128, 8], f32, tag="rdeg")
                nc.vector.reciprocal(rdeg[:], degc[:])

                # ---- proto_raw = S@(f/2) + ones x ch ; then /deg -> bf16 ----
                proto = ppool.tile([128, NT * 128], bf16, tag="proto")
                for h in range(2):
                    psp = pmm_pool.tile([128, 512], f32, tag="mm512")
                    for q in range(4):
                        mt = h * 4 + q
                        for kc in range(NT):
                            rhs = fh if kc < N_ACT else fv
                            nc.tensor.matmul(
                                psp[:, q * 128:(q + 1) * 128],
                                mask_t[:, kc * P + mt * 128: kc * P + (mt + 1) * 128],
                                rhs[:, kc * 128:(kc + 1) * 128],
                                start=(kc == 0),
                                stop=False,
                            )
                        nc.tensor.matmul(
                            psp[:, q * 128:(q + 1) * 128],
                            onesrow16[:],
                            ch[:],
                            start=False,
                            stop=False,
                        )
                        nc.tensor.matmul(
                            psp[:, q * 128:(q + 1) * 128],
                            onesrow16[:],
                            chlo[:],
                            start=False,
                            stop=True,
                        )
                    # normalize 4 m-tiles at once: proto = psp * rdeg (bcast)
                    nc.vector.scalar_tensor_tensor(
                        proto[:, h * 512:(h + 1) * 512].rearrange(
                            "p (q d) -> p q d", q=4),
                        psp[:].rearrange("p (q d) -> p q d", q=4),
                        1.0,
                        rdeg[:, h * 4:(h + 1) * 4].unsqueeze(2).broadcast_to(
                            [128, 4, 128]),
                        op0=OP.mult, op1=OP.mult,
                    )

                # ---- Sum proto*f and Sum proto^2, one DVE op each ----
                g1 = gscr.tile([128, NT * 128], bf16, tag="gscr")
                nc.vector.scalar_tensor_tensor(
                    g1[:], proto[:], 1.0, fb[:],
                    op0=OP.mult, op1=OP.mult, accum_out=statv[:, 8:9],
                )
                g2 = gscr.tile([128, NT * 128], bf16, tag="gscr")
                nc.vector.scalar_tensor_tensor(
                    g2[:], proto[:], 1.0, proto[:],
                    op0=OP.mult, op1=OP.mult, accum_out=statv[:, 9:10],
                )

                # ---- gtsum[d] = Sum_p proto[p, d] ----
                psg = pg_pool.tile([128, 128], f32, tag="psg")
                for mt in range(NT):
                    nc.tensor.matmul(
                        psg[0:1, :],
                        ones16[:, 0:1],
                        proto[:, mt * 128:(mt + 1) * 128],
                        start=(mt == 0),
                        stop=(mt == NT - 1),
                    )

                # ---- partition-sum the per-row stats ----
                pstat = pst_pool.tile([128, 128], f32, tag="pstat")
                nc.tensor.matmul(
                    pstat[0:1, 0:10], ones[:, 0:1], statv[:], start=True, stop=True
                )
                nc.scalar.copy(srow[0:1, s * ROW: s * ROW + 10], pstat[0:1, 0:10])
                nc.scalar.copy(srow[0:1, s * ROW + 32: s * ROW + 160], psg[0:1, :])

            nc.sync.dma_start(out_d[:], srow[:])

    nc.compile()
    return nc


def _get_program():
    global _PROG
    if _PROG is None:
        _PROG = _build_program()
    return _PROG


def _host_reduce(rows: np.ndarray) -> np.float32:
    """rows: [B, ROW] f32 per-sample device stats -> scalar loss."""
    rows = rows.astype(np.float64)
    N = float(P * D)
    ff = rows[:, 0:8].sum(axis=1)          # Sum f^2
    pf = rows[:, 8]                        # Sum proto*f
    pp = rows[:, 9]                        # Sum proto^2
    gtsum = rows[:, 32:160]                # Sum_p proto  [B, D]

    mse = (pp - 2.0 * pf + ff) / N
    sum_proto = gtsum.sum(axis=1)
    gtm = gtsum / float(P)
    ybar = sum_proto / N
    S = ((gtm - ybar[:, None]) ** 2).sum(axis=1)
    sum_xc2 = pp - (sum_proto ** 2) / N
    num = float(P) * S
    corr = num / np.sqrt(sum_xc2 * num)
    loss = mse.mean() + (0.5 * (corr + 1.0)).mean()
    return np.float32(loss)


_LAST_RESULTS = None


def kernel(features: np.ndarray, tau: np.ndarray, **run_kwargs) -> np.ndarray:
    global _LAST_RESULTS
    from concourse import bass_utils

    features = np.ascontiguousarray(features, dtype=np.float32)
    tau_v = np.array(tau, dtype=np.float32).reshape(1, 1)

    nc = _get_program()
    shards = features.reshape(NCORES, BLOC, P, D)
    in_maps = [
        {"features": shards[i], "tau": tau_v.copy()} for i in range(NCORES)
    ]
    res = bass_utils.run_bass_kernel_spmd(
        nc, in_maps, core_ids=list(range(NCORES)), **run_kwargs
    )
    _LAST_RESULTS = res
    rows = np.concatenate(
        [res.results[i]["out"].reshape(BLOC, ROW) for i in range(NCORES)], axis=0
    )
    return _host_reduce(rows)


if __name__ == "__main__":
    x = np.random.randn(B, P, D).astype(np.float32)
    t = np.float32(0.5)
    print(kernel(x, t))
